# revision 3
# baseline (speedup 1.0000x reference)
"""Distance-correlation (DcorLoss) kernel for 8 trn2 NeuronCores — v2.

Math: for x, y [n=8192, d=128]:
  a = pairwise_dist(x), b = pairwise_dist(y)   (n x n, symmetric, zero diag)
  dcor = -sqrt(sum(AB)) / sqrt(sqrt(sum(AA)) * sqrt(sum(BB)))
with A/B the double-centered matrices; all centering terms reduce to row
sums + aggregate scalars (host fp64), so the device only needs:
  rs_a[i] = sum_j a_ij,  rs_b[i],  pab = sum_ij (a_ij - mu) b_ij.

Device design (per core, block-row sharding rows [c*1024,(c+1)*1024)):
  - All operands are host-prepared fp8(e4m3). One DoubleRow fp8 matmul
    stream computes psum = n_i + n_j - 2 x_i.x_j directly: phys K rows
    0..63 carry the d=128 dims as DR pairs, rows 64..66 carry fp8
    hi/mid/lo splits of the row/col norms (rank-1 terms), rows 67..127
    are zero padding. 0.5 cycles/row => 4 matmuls ~107ns each per
    [128 x 2048] psum pair.
  - Columns are streamed in per-core ROTATED window order (window 0 ==
    the core's own diagonal block), so the mu^2*I diagonal fix (keeps
    sqrt NaN-free; diag lands exactly on bf16(16.0)) costs 2 extra
    matmuls only on the 8 jt==0 iterations.
  - One ACT pass per iteration: abT = sqrt(psum) over [128, 2048]
    (a and b windows adjacent in PSUM), bf16 out. ACT is the bottleneck
    engine (~0.83 ns/elem, dtype-independent).
  - DVE (all bf16 => 4x perf mode): two tensor_scalar copies with
    accum_out (row sums of a and b) + one scalar_tensor_tensor
    (aT-mu)*bT with accum_out (pab partials).
  - No on-device setup compute at all; outputs are the raw [128, 192]
    partial tables, combined on host in fp64.

Host finalize uses closed forms consistent with the exact device values
(fp8 operand bytes are bit-identical host/device): sum_ij dist^2 from
column sums of fp8 x and the exact split norms; diag contributes exactly
mu to each row sum and 0 to pab since sqrt(256 +- eps) rounds to bf16
16.0 (|eps| << half-ulp margin).
"""

import numpy as np
import ml_dtypes

import concourse.bass as bass
import concourse.tile as tile
from concourse import bacc, mybir
from concourse.bass_utils import run_bass_kernel_spmd

P = 128            # partitions / d
N = 8192           # points
NCORES = 8
BLK = N // NCORES  # 1024 rows per core
CI_N = BLK // P    # 8 row chunks per core
W = 1024           # column window
JT_N = N // W      # 8 column windows
NK = 128           # DR matmul phys K (67 used: 64 d-pairs + 3 norm rows)
MU = 16.0
F8 = ml_dtypes.float8_e4m3

_programs = {}


def _build():
    dt = mybir.dt
    f32 = dt.float32
    bf16 = dt.bfloat16
    f8 = dt.float8e4
    A = mybir.AluOpType
    AF = mybir.ActivationFunctionType
    DR = mybir.MatmulPerfMode.DoubleRow

    nc = bacc.Bacc("TRN2", target_bir_lowering=False, debug=False,
                   num_devices=NCORES)

    dWX = nc.dram_tensor("WX", [NK, 2, BLK], f8, kind="ExternalInput").ap()
    dWY = nc.dram_tensor("WY", [NK, 2, BLK], f8, kind="ExternalInput").ap()
    dMX = nc.dram_tensor("MX", [NK, 2, N], f8, kind="ExternalInput").ap()
    dMY = nc.dram_tensor("MY", [NK, 2, N], f8, kind="ExternalInput").ap()
    dEYE = nc.dram_tensor("EYE", [P, P], f8, kind="ExternalInput").ap()
    dEYW = nc.dram_tensor("EYW", [P, 4 * 512], f8, kind="ExternalInput").ap()
    dOUT = nc.dram_tensor("out", [P, 3 * CI_N * JT_N], f32,
                          kind="ExternalOutput").ap()

    with tile.TileContext(nc) as tc:
        with tc.tile_pool(name="const", bufs=1) as cp, \
             tc.tile_pool(name="psum", bufs=1, space="PSUM") as pp, \
             tc.tile_pool(name="ab", bufs=3) as abp, \
             tc.tile_pool(name="trd", bufs=2) as trd:

            wx = cp.tile([NK, 2, BLK], f8, tag="wx")
            wy = cp.tile([NK, 2, BLK], f8, tag="wy")
            mx = cp.tile([NK, 2, N], f8, tag="mx")
            my = cp.tile([NK, 2, N], f8, tag="my")
            eye = cp.tile([P, P], f8, tag="eye")
            eyw = cp.tile([P, 4 * 512], f8, tag="eyw")
            st = [cp.tile([P, CI_N * JT_N], f32, tag=f"st{q}", name=f"st{q}")
                  for q in range(3)]

            # PE warm-up on constant zeros (overlaps the input DMAs);
            # nudges the HAM clock-gate open before the real matmuls.
            wz = cp.tile([2, 512], f8, tag="wz")
            nc.vector.memset(wz[:], 0.0)
            wzl = cp.tile([2, P], f8, tag="wzl")
            nc.vector.memset(wzl[:], 0.0)
            for q in range(2):
                wt = pp.tile([P, 2 * W], f32, tag="ab", bufs=2)
                for h in range(4):
                    nc.tensor.matmul(wt[:, bass.ts(h, 512)], wzl[:], wz[:],
                                     start=True, stop=True)

            # input DMAs, first-needed first
            nc.sync.dma_start(wx[:], dWX[:])
            nc.sync.dma_start(wy[:], dWY[:])
            nc.sync.dma_start(eye[:], dEYE[:])
            nc.sync.dma_start(eyw[:], dEYW[:])
            for w in range(JT_N):
                sl = bass.ts(w, W)
                nc.sync.dma_start(mx[:, :, sl], dMX[:, :, sl])
                nc.sync.dma_start(my[:, :, sl], dMY[:, :, sl])

            # main loop: jt outer (window 0 first => diag + early DMA)
            for jt in range(JT_N):
                for ci in range(CI_N):
                    col = jt * CI_N + ci
                    hd = ci // 4
                    psAB = pp.tile([P, 2 * W], f32, tag="ab", bufs=2)
                    for base, wt_, mt_ in ((0, wx, mx), (W, wy, my)):
                        for h in range(2):
                            nc.tensor.matmul(
                                psAB[:, bass.ds(base + h * 512, 512)],
                                wt_[:, :, bass.ts(ci, P)],
                                mt_[:, :, bass.ds(jt * W + h * 512, 512)],
                                start=True,
                                stop=not (jt == 0 and h == hd),
                                perf_mode=DR)
                        if jt == 0:
                            nc.tensor.matmul(
                                psAB[:, bass.ds(base + hd * 512, 512)],
                                eye[:], eyw[:, bass.ts(ci % 4, 512)],
                                start=False, stop=True)

                    abT = abp.tile([P, 2 * W], f32, tag="ab")
                    nc.scalar.activation(abT[:, 0:W], psAB[:, 0:W], AF.Sqrt,
                                         accum_out=st[0][:, col:col + 1])
                    nc.scalar.activation(abT[:, W:2 * W], psAB[:, W:2 * W],
                                         AF.Sqrt,
                                         accum_out=st[1][:, col:col + 1])
                    t0 = trd.tile([P, W], f32, tag="r")
                    nc.vector.scalar_tensor_tensor(
                        t0[:], abT[:, 0:W], MU, abT[:, W:2 * W],
                        op0=A.subtract, op1=A.mult,
                        accum_out=st[2][:, col:col + 1])

            for q in range(3):
                nc.sync.dma_start(dOUT[:, bass.ts(q, CI_N * JT_N)], st[q][:])

    nc.compile()
    return nc


def _get_program():
    if "p" not in _programs:
        _programs["p"] = _build()
    return _programs["p"]


def _f8r(a):
    """Round to fp8 e4m3, return float64 view."""
    return np.asarray(a, np.float64).astype(F8).astype(np.float64)


def _prep(v):
    """Host-side fp8 prep for one tensor: returns dict of arrays/aggregates."""
    v8 = np.asarray(v, np.float32).astype(F8)          # fp8 bytes
    v8d = v8.astype(np.float64)
    assert np.all(np.isfinite(v8d))
    w8d = -2.0 * v8d                                   # exact in fp8
    nhat = (v8d * v8d).sum(1)                          # [N] fp64, exact
    hi = _f8r(nhat / 2.0)
    r1 = nhat - 2.0 * hi
    mid = _f8r(r1)
    r2 = r1 - mid
    lo = _f8r(r2)
    nspl = 2.0 * hi + mid + lo                         # device-added norms
    return dict(v8d=v8d, w8d=w8d, nhat=nhat, hi=hi, mid=mid, lo=lo,
                nspl=nspl)


def _pack_w(pr, rows):
    """Stationary tile [NK, 2, BLK] for a row block."""
    WT = np.zeros((NK, 2, BLK), np.float64)
    wb = pr["w8d"][rows]                               # [BLK, 128]
    for t in range(2):
        WT[0:64, t, :] = wb[:, t * 64:(t + 1) * 64].T
    WT[64, 0, :] = pr["hi"][rows]
    WT[64, 1, :] = pr["mid"][rows]
    WT[65, 0, :] = pr["lo"][rows]
    WT[65, 1, :] = 2.0
    WT[66, 0, :] = 1.0
    WT[66, 1, :] = 1.0
    return WT.astype(F8)


def _pack_m(pr, perm):
    """Moving tile [NK, 2, N] in rotated column order."""
    MT = np.zeros((NK, 2, N), np.float64)
    vp = pr["v8d"][perm]                               # [N, 128]
    for t in range(2):
        MT[0:64, t, :] = vp[:, t * 64:(t + 1) * 64].T
    MT[64, 0, :] = 2.0
    MT[64, 1, :] = 1.0
    MT[65, 0, :] = 1.0
    MT[65, 1, :] = pr["hi"][perm]
    MT[66, 0, :] = pr["mid"][perm]
    MT[66, 1, :] = pr["lo"][perm]
    return MT.astype(F8)


def make_in_maps(x, y):
    px = _prep(x)
    py = _prep(y)
    eye = (np.eye(P) * MU).astype(F8)
    eyw = np.zeros((P, 4 * 512), np.float64)
    for k in range(4):
        for p in range(P):
            eyw[p, k * 512 + k * P + p] = MU
    eyw = eyw.astype(F8)
    in_maps = []
    for c in range(NCORES):
        rows = slice(c * BLK, (c + 1) * BLK)
        perm = np.concatenate(
            [np.arange(((c + w) % JT_N) * W, (((c + w) % JT_N) + 1) * W)
             for w in range(JT_N)])
        in_maps.append({
            "WX": _pack_w(px, rows),
            "WY": _pack_w(py, rows),
            "MX": _pack_m(px, perm),
            "MY": _pack_m(py, perm),
            "EYE": eye,
            "EYW": eyw,
        })
    return in_maps, (px, py)


def finalize(outs, px, py):
    """outs: 8 x [128, 192] f32 -> scalar dcor (fp64 host math).

    out cols: [0:64] rs_a partials, [64:128] rs_b, [128:192] pab,
    col index = jt*8 + ci, partition p = row c*1024 + ci*128 + p.
    """
    n = float(N)
    rs_a = np.empty(N, np.float64)
    rs_b = np.empty(N, np.float64)
    pab = 0.0
    for c, o in enumerate(outs):
        o = np.asarray(o, np.float64)
        for q, dst in ((0, rs_a), (1, rs_b)):
            part = o[:, q * 64:(q + 1) * 64]           # [128, 64]
            per_ci = part.reshape(P, JT_N, CI_N).sum(axis=1)  # [128, 8]
            dst[c * BLK:(c + 1) * BLK] = per_ci.T.ravel()
        pab += o[:, 128:192].sum()

    def sq_closed(pr):
        sx = pr["v8d"].sum(0)                          # [d]
        Sg = float((sx * sx).sum())
        q_ii = 256.0 + 2.0 * (pr["nspl"] - pr["nhat"])
        return (2.0 * n * pr["nspl"].sum() - 2.0 * Sg + 256.0 * n
                - q_ii.sum())

    sq_a = sq_closed(px)
    sq_b = sq_closed(py)

    sa = rs_a - MU          # diag is exactly bf16(16.0)
    sb = rs_b - MU
    sat = sa - n * MU
    sbt = sb - n * MU
    Ua = sat.sum()
    Ub = sbt.sum()
    Sab = pab - MU * (sa.sum() - MU * n * n)
    Saa = sq_a - 2.0 * MU * sa.sum() + MU * MU * n * n
    Sbb = sq_b - 2.0 * MU * sb.sum() + MU * MU * n * n

    sumAB = Sab - 2.0 * np.dot(sat, sbt) / n + Ua * Ub / n ** 2
    sumAA = Saa - 2.0 * np.dot(sat, sat) / n + Ua * Ua / n ** 2
    sumBB = Sbb - 2.0 * np.dot(sbt, sbt) / n + Ub * Ub / n ** 2

    inv_n2 = 1.0 / (n * n)
    dcor = (-np.sqrt(sumAB * inv_n2)
            / np.sqrt(np.sqrt(sumAA * inv_n2) * np.sqrt(sumBB * inv_n2)))
    return np.asarray(dcor, dtype=np.float32)


def run(x, y, mm_mode=None, trace=False, tmpdir=None):
    nc = _get_program()
    in_maps, (px, py) = make_in_maps(x, y)
    res = run_bass_kernel_spmd(nc, in_maps, core_ids=list(range(NCORES)),
                               trace=trace, tmpdir=tmpdir)
    outs = [r["out"] for r in res.results]
    return finalize(outs, px, py), res


def kernel(x, y):
    val, _ = run(x, y)
    return val


# revision 4
# speedup vs baseline: 1.0427x; 1.0427x over previous
"""DcorLoss kernel v3 — symmetric-triangle variant.

Same fp8-DoubleRow psum math as v2 (psum = n_i + n_j - 2 x_i.x_j via one
DR matmul stream; mu^2*I added on diagonal chunks; a = sqrt(psum)), but
exploits symmetry of the distance matrices: only 36 of 64 [128 x 1024]
cells per core are computed.

Cell decomposition: 64 row-chunks (I) x 8 col-windows (Jw). Each core c:
  - 8 "diag" cells: I = 8c+ci, window c  (rowsums only; the off-diagonal
    128-chunks inside the diagonal block pair up across cells, so
    counting rowsums once covers them exactly).
  - 28 "upper" cells from a 7-round round-robin tournament of the 8
    blocks: round pairs (m, M), rows from block m, window M; the two
    partner cores take 4 cells each. Counted twice (pab) and both
    rowsums (ACT accum) + colsums (PE f32r ones-matmul into PSUM,
    drained per round) feed the global row sums.

Engine budget per cell: ACT 2x(sqrt[128,1024]+accum) ~2.7us (bottleneck),
PE ~1.3-2.1us, DVE (STT pab + psC drains) ~1.4us.
"""

import numpy as np
import ml_dtypes

import concourse.bass as bass
import concourse.tile as tile
from concourse import bacc, mybir
from concourse.bass_utils import run_bass_kernel_spmd

P = 128
N = 8192
NCORES = 8
BLK = 1024
CI_N = 8
W = 1024
JT_N = 8
NK = 128
MU = 16.0
F8 = ml_dtypes.float8_e4m3
NCELL = 36          # 8 diag + 28 upper
NROUND = 7

_programs = {}


def _partner(c, r):
    """Round-robin circle method for 8 teams, rounds 0..6."""
    if c == 7:
        return r
    if r == c:
        return 7
    return (2 * r - c) % 7


def _schedule(c):
    """Per-core cell list: (wx_slot, win_slot, kind, round, k).

    kind: 'diag' or 'upper'. wx_slot: 0..35 into the stationary tile.
    win_slot: 0..7 into the moving tile. Mirrors on host and device.
    """
    cells = []
    for ci in range(CI_N):
        cells.append((ci, 0, "diag", None, ci))
    for r in range(NROUND):
        for k in range(4):
            cells.append((8 + 4 * r + k, r + 1, "upper", r, k))
    return cells


def _wx_chunks(c):
    """Global row-chunk index for each of the 36 stationary slots."""
    chunks = [8 * c + s for s in range(8)]
    for r in range(NROUND):
        p = _partner(c, r)
        m = min(c, p)
        for k in range(4):
            off = k if c == m else 4 + k
            chunks.append(8 * m + off)
    return chunks


def _windows(c):
    """Global window index for each of the 8 moving slots."""
    wins = [c]
    for r in range(NROUND):
        wins.append(max(c, _partner(c, r)))
    return wins


def _build():
    dt = mybir.dt
    f32 = dt.float32
    f32r = dt.float32r
    f8 = dt.float8e4
    A = mybir.AluOpType
    AF = mybir.ActivationFunctionType
    DR = mybir.MatmulPerfMode.DoubleRow

    nc = bacc.Bacc("TRN2", target_bir_lowering=False, debug=False,
                   num_devices=NCORES)

    dWX = nc.dram_tensor("WX", [NK, 2, NCELL * P], f8,
                         kind="ExternalInput").ap()
    dWY = nc.dram_tensor("WY", [NK, 2, NCELL * P], f8,
                         kind="ExternalInput").ap()
    dMX = nc.dram_tensor("MX", [NK, 2, N], f8, kind="ExternalInput").ap()
    dMY = nc.dram_tensor("MY", [NK, 2, N], f8, kind="ExternalInput").ap()
    dEYE = nc.dram_tensor("EYE", [P, P], f8, kind="ExternalInput").ap()
    dEYW = nc.dram_tensor("EYW", [P, 4 * 512], f8, kind="ExternalInput").ap()
    dSEL = nc.dram_tensor("SEL", [P, 16], mybir.dt.float32r,
                          kind="ExternalInput").ap()
    dOUT = nc.dram_tensor("out", [P, 3 * NCELL], f32,
                          kind="ExternalOutput").ap()
    dCS = nc.dram_tensor("cs", [4, NROUND * 512], f32,
                         kind="ExternalOutput").ap()

    cells = _schedule(0)   # slot structure is identical on every core

    with tile.TileContext(nc) as tc:
        with tc.tile_pool(name="const", bufs=1) as cp, \
             tc.tile_pool(name="psum", bufs=1, space="PSUM") as pp, \
             tc.tile_pool(name="ab", bufs=4) as abp, \
             tc.tile_pool(name="trd", bufs=2) as trd:

            wx = cp.tile([NK, 2, NCELL * P], f8, tag="wx")
            wy = cp.tile([NK, 2, NCELL * P], f8, tag="wy")
            mx = cp.tile([NK, 2, N], f8, tag="mx")
            my = cp.tile([NK, 2, N], f8, tag="my")
            eye = cp.tile([P, P], f8, tag="eye")
            eyw = cp.tile([P, 4 * 512], f8, tag="eyw")
            # sel[:, 4q+j] = (j == q): ones-selector weights so colsum q
            # lands on psum partition q (matmul base partition must be 0)
            sel = cp.tile([P, 16], f32r, tag="sel")
            colsb = cp.tile([4, NROUND * 512], f32, tag="colsb")
            st = [cp.tile([P, NCELL], f32, tag=f"st{q}", name=f"st{q}")
                  for q in range(3)]

            # diag cells (slots 0-7, window 0) first: ship only what they
            # need on the sync queue; stream the rest from the idle gpsimd
            # queue behind the compute
            s07 = bass.ds(0, 8 * P)
            nc.sync.dma_start(wx[:, :, s07], dWX[:, :, s07])
            nc.sync.dma_start(wy[:, :, s07], dWY[:, :, s07])
            sl0 = bass.ts(0, W)
            nc.sync.dma_start(mx[:, :, sl0], dMX[:, :, sl0])
            nc.sync.dma_start(my[:, :, sl0], dMY[:, :, sl0])
            nc.sync.dma_start(eye[:], dEYE[:])
            nc.sync.dma_start(eyw[:], dEYW[:])
            nc.sync.dma_start(sel[:], dSEL[:])
            srest = bass.ds(8 * P, (NCELL - 8) * P)
            nc.sync.dma_start(wx[:, :, srest], dWX[:, :, srest])
            nc.sync.dma_start(wy[:, :, srest], dWY[:, :, srest])
            for w in range(1, JT_N):
                sl = bass.ts(w, W)
                nc.sync.dma_start(mx[:, :, sl], dMX[:, :, sl])
                nc.sync.dma_start(my[:, :, sl], dMY[:, :, sl])

            wz = cp.tile([2, 512], f8, tag="wz")
            nc.vector.memset(wz[:], 0.0)
            wzl = cp.tile([2, P], f8, tag="wzl")
            nc.vector.memset(wzl[:], 0.0)
            for q in range(3):
                wt = pp.tile([P, W], f32, tag="ps", bufs=3)
                for h in range(2):
                    nc.tensor.matmul(wt[:, bass.ts(h, 512)], wzl[:], wz[:],
                                     start=True, stop=True)

            pend = []       # delayed colsum work: (aT, bT, rnd, k)
            pc = {"t": None}

            def emit_colsums():
                if not pend:
                    return
                aT, bT, rnd, k = pend.pop(0)
                if k == 0:
                    pc["t"] = pp.tile([4, 512], f32, tag="pc", bufs=2,
                                      name="pc")
                pct = pc["t"]
                for q, (src, h) in enumerate(
                        ((aT, 0), (aT, 1), (bT, 0), (bT, 1))):
                    nc.tensor.matmul(
                        pct[:, :],
                        sel[:, bass.ts(q, 4)],
                        src[:, bass.ts(h, 512)],
                        start=(k == 0 and q == 0),
                        stop=(k == 3 and q == 3))
                if k == 3:
                    nc.vector.tensor_copy(
                        colsb[:, bass.ts(rnd, 512)], pct[:, :])

            for idx, (ss, ws, kind, rnd, k) in enumerate(cells):
                psA = pp.tile([P, W], f32, tag="ps", bufs=3)
                psB = pp.tile([P, W], f32, tag="ps", bufs=3)
                diag = kind == "diag"
                hd = k // 4 if not diag else k // 4  # k==ci for diag
                ci = k
                for ps_, wt_, mt_ in ((psA, wx, mx), (psB, wy, my)):
                    for h in range(2):
                        nc.tensor.matmul(
                            ps_[:, bass.ts(h, 512)],
                            wt_[:, :, bass.ts(ss, P)],
                            mt_[:, :, bass.ds(ws * W + h * 512, 512)],
                            start=True,
                            stop=not (diag and h == ci // 4),
                            perf_mode=DR)
                    if diag:
                        nc.tensor.matmul(
                            ps_[:, bass.ts(ci // 4, 512)],
                            eye[:], eyw[:, bass.ts(ci % 4, 512)],
                            start=False, stop=True)
                emit_colsums()

                aT = abp.tile([P, W], f32r, tag="ab")
                bT = abp.tile([P, W], f32r, tag="ab")
                nc.scalar.activation(aT[:], psA[:], AF.Sqrt,
                                     accum_out=st[0][:, idx:idx + 1])
                nc.scalar.activation(bT[:], psB[:], AF.Sqrt,
                                     accum_out=st[1][:, idx:idx + 1])
                t0 = trd.tile([P, W], f32, tag="r")
                nc.vector.scalar_tensor_tensor(
                    t0[:], aT[:].bitcast(f32), MU, bT[:].bitcast(f32),
                    op0=A.subtract, op1=A.mult,
                    accum_out=st[2][:, idx:idx + 1])
                if not diag:
                    pend.append((aT, bT, rnd, k))

            while pend:
                emit_colsums()

            for q in range(3):
                nc.sync.dma_start(dOUT[:, bass.ts(q, NCELL)], st[q][:])
            nc.sync.dma_start(dCS[:], colsb[:])

    nc.compile()
    return nc


def _get_program():
    if "p" not in _programs:
        _programs["p"] = _build()
    return _programs["p"]


def _f8r(a):
    return np.asarray(a, np.float64).astype(F8).astype(np.float64)


def _prep(v):
    v8 = np.asarray(v, np.float32).astype(F8)
    v8d = v8.astype(np.float64)
    assert np.all(np.isfinite(v8d))
    w8d = -2.0 * v8d
    nhat = (v8d * v8d).sum(1)
    hi = _f8r(nhat / 2.0)
    r1 = nhat - 2.0 * hi
    mid = _f8r(r1)
    r2 = r1 - mid
    lo = _f8r(r2)
    nspl = 2.0 * hi + mid + lo
    return dict(v8d=v8d, w8d=w8d, nhat=nhat, hi=hi, mid=mid, lo=lo,
                nspl=nspl)


def _pack_w(pr, row_idx):
    """Stationary tile [NK, 2, NCELL*P] for given global rows."""
    nslot = len(row_idx) // P
    WT = np.zeros((NK, 2, nslot * P), np.float64)
    wb = pr["w8d"][row_idx]
    for t in range(2):
        WT[0:64, t, :] = wb[:, t * 64:(t + 1) * 64].T
    WT[64, 0, :] = pr["hi"][row_idx]
    WT[64, 1, :] = pr["mid"][row_idx]
    WT[65, 0, :] = pr["lo"][row_idx]
    WT[65, 1, :] = 2.0
    WT[66, 0, :] = 1.0
    WT[66, 1, :] = 1.0
    return WT.astype(F8)


def _pack_m(pr, perm):
    MT = np.zeros((NK, 2, N), np.float64)
    vp = pr["v8d"][perm]
    for t in range(2):
        MT[0:64, t, :] = vp[:, t * 64:(t + 1) * 64].T
    MT[64, 0, :] = 2.0
    MT[64, 1, :] = 1.0
    MT[65, 0, :] = 1.0
    MT[65, 1, :] = pr["hi"][perm]
    MT[66, 0, :] = pr["mid"][perm]
    MT[66, 1, :] = pr["lo"][perm]
    return MT.astype(F8)


def make_in_maps(x, y):
    px = _prep(x)
    py = _prep(y)
    eye = (np.eye(P) * MU).astype(F8)
    eyw = np.zeros((P, 4 * 512), np.float64)
    for kk in range(4):
        for p in range(P):
            eyw[p, kk * 512 + kk * P + p] = MU
    eyw = eyw.astype(F8)
    selh = np.zeros((P, 16), np.float32)
    for q in range(4):
        selh[:, 5 * q] = 1.0
    in_maps = []
    for c in range(NCORES):
        chunks = _wx_chunks(c)
        row_idx = np.concatenate(
            [np.arange(I * P, (I + 1) * P) for I in chunks])
        wins = _windows(c)
        perm = np.concatenate(
            [np.arange(wv * W, (wv + 1) * W) for wv in wins])
        in_maps.append({
            "WX": _pack_w(px, row_idx),
            "WY": _pack_w(py, row_idx),
            "MX": _pack_m(px, perm),
            "MY": _pack_m(py, perm),
            "EYE": eye,
            "EYW": eyw,
            "SEL": selh,
        })
    return in_maps, (px, py)


def finalize(results, px, py):
    n = float(N)
    rs_a = np.zeros(N, np.float64)
    rs_b = np.zeros(N, np.float64)
    pab = 0.0
    for c in range(NCORES):
        o = np.asarray(results[c]["out"], np.float64)
        cs = np.asarray(results[c]["cs"], np.float64)
        cells = _schedule(c)
        chunks = _wx_chunks(c)
        wins = _windows(c)
        for idx, (ss, ws, kind, rnd, k) in enumerate(cells):
            I = chunks[ss]
            rows = slice(I * P, (I + 1) * P)
            rs_a[rows] += o[:, idx]
            rs_b[rows] += o[:, NCELL + idx]
            mult = 1.0 if kind == "diag" else 2.0
            pab += mult * o[:, 2 * NCELL + idx].sum()
        # colsums: round r covers window wins[r+1]
        for r in range(NROUND):
            wv = wins[r + 1]
            seg = cs[:, r * 512:(r + 1) * 512]
            rs_a[wv * W:wv * W + 512] += seg[0]
            rs_a[wv * W + 512:(wv + 1) * W] += seg[1]
            rs_b[wv * W:wv * W + 512] += seg[2]
            rs_b[wv * W + 512:(wv + 1) * W] += seg[3]

    def sq_closed(pr):
        sx = pr["v8d"].sum(0)
        Sg = float((sx * sx).sum())
        q_ii = 256.0 + 2.0 * (pr["nspl"] - pr["nhat"])
        return (2.0 * n * pr["nspl"].sum() - 2.0 * Sg + 256.0 * n
                - q_ii.sum())

    sq_a = sq_closed(px)
    sq_b = sq_closed(py)

    sa = rs_a - MU
    sb = rs_b - MU
    sat = sa - n * MU
    sbt = sb - n * MU
    Ua = sat.sum()
    Ub = sbt.sum()
    Sab = pab - MU * (sa.sum() - MU * n * n)
    Saa = sq_a - 2.0 * MU * sa.sum() + MU * MU * n * n
    Sbb = sq_b - 2.0 * MU * sb.sum() + MU * MU * n * n

    sumAB = Sab - 2.0 * np.dot(sat, sbt) / n + Ua * Ub / n ** 2
    sumAA = Saa - 2.0 * np.dot(sat, sat) / n + Ua * Ua / n ** 2
    sumBB = Sbb - 2.0 * np.dot(sbt, sbt) / n + Ub * Ub / n ** 2

    inv_n2 = 1.0 / (n * n)
    dcor = (-np.sqrt(sumAB * inv_n2)
            / np.sqrt(np.sqrt(sumAA * inv_n2) * np.sqrt(sumBB * inv_n2)))
    return np.asarray(dcor, dtype=np.float32)


def run(x, y, mm_mode=None, trace=False, tmpdir=None):
    nc = _get_program()
    in_maps, (px, py) = make_in_maps(x, y)
    res = run_bass_kernel_spmd(nc, in_maps, core_ids=list(range(NCORES)),
                               trace=trace, tmpdir=tmpdir)
    return finalize(res.results, px, py), res


def kernel(x, y):
    val, _ = run(x, y)
    return val


# revision 5
# speedup vs baseline: 1.0503x; 1.0074x over previous
"""DcorLoss kernel v3 — symmetric-triangle variant.

Same fp8-DoubleRow psum math as v2 (psum = n_i + n_j - 2 x_i.x_j via one
DR matmul stream; mu^2*I added on diagonal chunks; a = sqrt(psum)), but
exploits symmetry of the distance matrices: only 36 of 64 [128 x 1024]
cells per core are computed.

Cell decomposition: 64 row-chunks (I) x 8 col-windows (Jw). Each core c:
  - 8 "diag" cells: I = 8c+ci, window c  (rowsums only; the off-diagonal
    128-chunks inside the diagonal block pair up across cells, so
    counting rowsums once covers them exactly).
  - 28 "upper" cells from a 7-round round-robin tournament of the 8
    blocks: round pairs (m, M), rows from block m, window M; the two
    partner cores take 4 cells each. Counted twice (pab) and both
    rowsums (ACT accum) + colsums (PE f32r ones-matmul into PSUM,
    drained per round) feed the global row sums.

Engine budget per cell: ACT 2x(sqrt[128,1024]+accum) ~2.7us (bottleneck),
PE ~1.3-2.1us, DVE (STT pab + psC drains) ~1.4us.
"""

import numpy as np
import ml_dtypes

import concourse.bass as bass
import concourse.tile as tile
from concourse import bacc, mybir
from concourse.bass_utils import run_bass_kernel_spmd

P = 128
N = 8192
NCORES = 8
BLK = 1024
CI_N = 8
W = 1024
JT_N = 8
NK = 128
MU = 16.0
F8 = ml_dtypes.float8_e4m3
NCELL = 36          # 8 diag + 28 upper
NROUND = 7

_programs = {}


def _partner(c, r):
    """Round-robin circle method for 8 teams, rounds 0..6."""
    if c == 7:
        return r
    if r == c:
        return 7
    return (2 * r - c) % 7


def _schedule(c):
    """Per-core cell list: (wx_slot, win_slot, kind, round, k).

    kind: 'diag' or 'upper'. wx_slot: 0..35 into the stationary tile.
    win_slot: 0..7 into the moving tile. Mirrors on host and device.
    """
    cells = []
    for r in range(NROUND):
        for k in range(4):
            cells.append((8 + 4 * r + k, r + 1, "upper", r, k))
    # diag cells last: the kernel tail then has no colsum chain, and the
    # final round's psC drain overlaps the diag phase
    for ci in range(CI_N):
        cells.append((ci, 0, "diag", None, ci))
    return cells


def _wx_chunks(c):
    """Global row-chunk index for each of the 36 stationary slots."""
    chunks = [8 * c + s for s in range(8)]
    for r in range(NROUND):
        p = _partner(c, r)
        m = min(c, p)
        for k in range(4):
            off = k if c == m else 4 + k
            chunks.append(8 * m + off)
    return chunks


def _windows(c):
    """Global window index for each of the 8 moving slots."""
    wins = [c]
    for r in range(NROUND):
        wins.append(max(c, _partner(c, r)))
    return wins


def _build():
    dt = mybir.dt
    f32 = dt.float32
    f32r = dt.float32r
    f8 = dt.float8e4
    A = mybir.AluOpType
    AF = mybir.ActivationFunctionType
    DR = mybir.MatmulPerfMode.DoubleRow

    nc = bacc.Bacc("TRN2", target_bir_lowering=False, debug=False,
                   num_devices=NCORES)

    dWX = nc.dram_tensor("WX", [NK, 2, NCELL * P], f8,
                         kind="ExternalInput").ap()
    dWY = nc.dram_tensor("WY", [NK, 2, NCELL * P], f8,
                         kind="ExternalInput").ap()
    dMX = nc.dram_tensor("MX", [NK, 2, N], f8, kind="ExternalInput").ap()
    dMY = nc.dram_tensor("MY", [NK, 2, N], f8, kind="ExternalInput").ap()
    dEYE = nc.dram_tensor("EYE", [P, P], f8, kind="ExternalInput").ap()
    dEYW = nc.dram_tensor("EYW", [P, 4 * 512], f8, kind="ExternalInput").ap()
    dSEL = nc.dram_tensor("SEL", [P, 16], mybir.dt.float32r,
                          kind="ExternalInput").ap()
    dOUT = nc.dram_tensor("out", [P, 3 * NCELL], f32,
                          kind="ExternalOutput").ap()
    dCS = nc.dram_tensor("cs", [4, NROUND * 512], f32,
                         kind="ExternalOutput").ap()

    cells = _schedule(0)   # slot structure is identical on every core

    with tile.TileContext(nc) as tc:
        with tc.tile_pool(name="const", bufs=1) as cp, \
             tc.tile_pool(name="psum", bufs=1, space="PSUM") as pp, \
             tc.tile_pool(name="ab", bufs=4) as abp, \
             tc.tile_pool(name="trd", bufs=2) as trd:

            wx = cp.tile([NK, 2, NCELL * P], f8, tag="wx")
            wy = cp.tile([NK, 2, NCELL * P], f8, tag="wy")
            mx = cp.tile([NK, 2, N], f8, tag="mx")
            my = cp.tile([NK, 2, N], f8, tag="my")
            eye = cp.tile([P, P], f8, tag="eye")
            eyw = cp.tile([P, 4 * 512], f8, tag="eyw")
            # sel[:, 4q+j] = (j == q): ones-selector weights so colsum q
            # lands on psum partition q (matmul base partition must be 0)
            sel = cp.tile([P, 16], f32r, tag="sel")
            colsb = cp.tile([4, NROUND * 512], f32, tag="colsb")
            st = [cp.tile([P, NCELL], f32, tag=f"st{q}", name=f"st{q}")
                  for q in range(3)]

            # diag cells (slots 0-7, window 0) first: ship only what they
            # need on the sync queue; stream the rest from the idle gpsimd
            # queue behind the compute
            # round-0 cells (slots 8-11, window 1) run first: ship their
            # data first, split across two queues to parallelize the
            # ~0.6us-per-DMA issue latency; diag-phase data (slots 0-7,
            # window 0, eye/eyw) is needed last
            s811 = bass.ds(8 * P, 8 * P)
            sl1 = bass.ts(1, W)
            nc.sync.dma_start(wx[:, :, s811], dWX[:, :, s811])
            nc.scalar.dma_start(wy[:, :, s811], dWY[:, :, s811])
            nc.sync.dma_start(mx[:, :, sl1], dMX[:, :, sl1])
            nc.scalar.dma_start(my[:, :, sl1], dMY[:, :, sl1])
            nc.sync.dma_start(sel[:], dSEL[:])
            for w in range(2, JT_N):
                sl = bass.ts(w, W)
                nc.sync.dma_start(mx[:, :, sl], dMX[:, :, sl])
                nc.sync.dma_start(my[:, :, sl], dMY[:, :, sl])
            srest = bass.ds(16 * P, (NCELL - 16) * P)
            nc.sync.dma_start(wx[:, :, srest], dWX[:, :, srest])
            nc.sync.dma_start(wy[:, :, srest], dWY[:, :, srest])
            s07 = bass.ds(0, 8 * P)
            sl0 = bass.ts(0, W)
            nc.sync.dma_start(wx[:, :, s07], dWX[:, :, s07])
            nc.sync.dma_start(wy[:, :, s07], dWY[:, :, s07])
            nc.sync.dma_start(mx[:, :, sl0], dMX[:, :, sl0])
            nc.sync.dma_start(my[:, :, sl0], dMY[:, :, sl0])
            nc.sync.dma_start(eye[:], dEYE[:])
            nc.sync.dma_start(eyw[:], dEYW[:])

            wz = cp.tile([2, 512], f8, tag="wz")
            nc.vector.memset(wz[:], 0.0)
            wzl = cp.tile([2, P], f8, tag="wzl")
            nc.vector.memset(wzl[:], 0.0)
            for q in range(3):
                wt = pp.tile([P, W], f32, tag="ps", bufs=3)
                for h in range(2):
                    nc.tensor.matmul(wt[:, bass.ts(h, 512)], wzl[:], wz[:],
                                     start=True, stop=True)

            pend = []       # delayed colsum work: (aT, bT, rnd, k)
            pc = {"t": None}

            def emit_colsums():
                if not pend:
                    return
                aT, bT, rnd, k = pend.pop(0)
                if k == 0:
                    pc["t"] = pp.tile([4, 512], f32, tag="pc", bufs=2,
                                      name="pc")
                pct = pc["t"]
                for q, (src, h) in enumerate(
                        ((aT, 0), (aT, 1), (bT, 0), (bT, 1))):
                    nc.tensor.matmul(
                        pct[:, :],
                        sel[:, bass.ts(q, 4)],
                        src[:, bass.ts(h, 512)],
                        start=(k == 0 and q == 0),
                        stop=(k == 3 and q == 3))
                if k == 3:
                    nc.vector.tensor_copy(
                        colsb[:, bass.ts(rnd, 512)], pct[:, :])

            for idx, (ss, ws, kind, rnd, k) in enumerate(cells):
                psA = pp.tile([P, W], f32, tag="ps", bufs=3)
                psB = pp.tile([P, W], f32, tag="ps", bufs=3)
                diag = kind == "diag"
                hd = k // 4 if not diag else k // 4  # k==ci for diag
                ci = k
                for ps_, wt_, mt_ in ((psA, wx, mx), (psB, wy, my)):
                    for h in range(2):
                        nc.tensor.matmul(
                            ps_[:, bass.ts(h, 512)],
                            wt_[:, :, bass.ts(ss, P)],
                            mt_[:, :, bass.ds(ws * W + h * 512, 512)],
                            start=True,
                            stop=not (diag and h == ci // 4),
                            perf_mode=DR)
                    if diag:
                        nc.tensor.matmul(
                            ps_[:, bass.ts(ci // 4, 512)],
                            eye[:], eyw[:, bass.ts(ci % 4, 512)],
                            start=False, stop=True)
                emit_colsums()

                aT = abp.tile([P, W], f32r, tag="ab")
                bT = abp.tile([P, W], f32r, tag="ab")
                nc.scalar.activation(aT[:], psA[:], AF.Sqrt,
                                     accum_out=st[0][:, idx:idx + 1])
                nc.scalar.activation(bT[:], psB[:], AF.Sqrt,
                                     accum_out=st[1][:, idx:idx + 1])
                t0 = trd.tile([P, W], f32, tag="r")
                nc.vector.scalar_tensor_tensor(
                    t0[:], aT[:].bitcast(f32), MU, bT[:].bitcast(f32),
                    op0=A.subtract, op1=A.mult,
                    accum_out=st[2][:, idx:idx + 1])
                if not diag:
                    pend.append((aT, bT, rnd, k))

            while pend:
                emit_colsums()

            for q in range(3):
                nc.sync.dma_start(dOUT[:, bass.ts(q, NCELL)], st[q][:])
            nc.sync.dma_start(dCS[:], colsb[:])

    nc.compile()
    return nc


def _get_program():
    if "p" not in _programs:
        _programs["p"] = _build()
    return _programs["p"]


def _f8r(a):
    return np.asarray(a, np.float64).astype(F8).astype(np.float64)


def _prep(v):
    v8 = np.asarray(v, np.float32).astype(F8)
    v8d = v8.astype(np.float64)
    assert np.all(np.isfinite(v8d))
    w8d = -2.0 * v8d
    nhat = (v8d * v8d).sum(1)
    hi = _f8r(nhat / 2.0)
    r1 = nhat - 2.0 * hi
    mid = _f8r(r1)
    r2 = r1 - mid
    lo = _f8r(r2)
    nspl = 2.0 * hi + mid + lo
    return dict(v8d=v8d, w8d=w8d, nhat=nhat, hi=hi, mid=mid, lo=lo,
                nspl=nspl)


def _pack_w(pr, row_idx):
    """Stationary tile [NK, 2, NCELL*P] for given global rows."""
    nslot = len(row_idx) // P
    WT = np.zeros((NK, 2, nslot * P), np.float64)
    wb = pr["w8d"][row_idx]
    for t in range(2):
        WT[0:64, t, :] = wb[:, t * 64:(t + 1) * 64].T
    WT[64, 0, :] = pr["hi"][row_idx]
    WT[64, 1, :] = pr["mid"][row_idx]
    WT[65, 0, :] = pr["lo"][row_idx]
    WT[65, 1, :] = 2.0
    WT[66, 0, :] = 1.0
    WT[66, 1, :] = 1.0
    return WT.astype(F8)


def _pack_m(pr, perm):
    MT = np.zeros((NK, 2, N), np.float64)
    vp = pr["v8d"][perm]
    for t in range(2):
        MT[0:64, t, :] = vp[:, t * 64:(t + 1) * 64].T
    MT[64, 0, :] = 2.0
    MT[64, 1, :] = 1.0
    MT[65, 0, :] = 1.0
    MT[65, 1, :] = pr["hi"][perm]
    MT[66, 0, :] = pr["mid"][perm]
    MT[66, 1, :] = pr["lo"][perm]
    return MT.astype(F8)


def make_in_maps(x, y):
    px = _prep(x)
    py = _prep(y)
    eye = (np.eye(P) * MU).astype(F8)
    eyw = np.zeros((P, 4 * 512), np.float64)
    for kk in range(4):
        for p in range(P):
            eyw[p, kk * 512 + kk * P + p] = MU
    eyw = eyw.astype(F8)
    selh = np.zeros((P, 16), np.float32)
    for q in range(4):
        selh[:, 5 * q] = 1.0
    in_maps = []
    for c in range(NCORES):
        chunks = _wx_chunks(c)
        row_idx = np.concatenate(
            [np.arange(I * P, (I + 1) * P) for I in chunks])
        wins = _windows(c)
        perm = np.concatenate(
            [np.arange(wv * W, (wv + 1) * W) for wv in wins])
        in_maps.append({
            "WX": _pack_w(px, row_idx),
            "WY": _pack_w(py, row_idx),
            "MX": _pack_m(px, perm),
            "MY": _pack_m(py, perm),
            "EYE": eye,
            "EYW": eyw,
            "SEL": selh,
        })
    return in_maps, (px, py)


def finalize(results, px, py):
    n = float(N)
    rs_a = np.zeros(N, np.float64)
    rs_b = np.zeros(N, np.float64)
    pab = 0.0
    for c in range(NCORES):
        o = np.asarray(results[c]["out"], np.float64)
        cs = np.asarray(results[c]["cs"], np.float64)
        cells = _schedule(c)
        chunks = _wx_chunks(c)
        wins = _windows(c)
        for idx, (ss, ws, kind, rnd, k) in enumerate(cells):
            I = chunks[ss]
            rows = slice(I * P, (I + 1) * P)
            rs_a[rows] += o[:, idx]
            rs_b[rows] += o[:, NCELL + idx]
            mult = 1.0 if kind == "diag" else 2.0
            pab += mult * o[:, 2 * NCELL + idx].sum()
        # colsums: round r covers window wins[r+1]
        for r in range(NROUND):
            wv = wins[r + 1]
            seg = cs[:, r * 512:(r + 1) * 512]
            rs_a[wv * W:wv * W + 512] += seg[0]
            rs_a[wv * W + 512:(wv + 1) * W] += seg[1]
            rs_b[wv * W:wv * W + 512] += seg[2]
            rs_b[wv * W + 512:(wv + 1) * W] += seg[3]

    def sq_closed(pr):
        sx = pr["v8d"].sum(0)
        Sg = float((sx * sx).sum())
        q_ii = 256.0 + 2.0 * (pr["nspl"] - pr["nhat"])
        return (2.0 * n * pr["nspl"].sum() - 2.0 * Sg + 256.0 * n
                - q_ii.sum())

    sq_a = sq_closed(px)
    sq_b = sq_closed(py)

    sa = rs_a - MU
    sb = rs_b - MU
    sat = sa - n * MU
    sbt = sb - n * MU
    Ua = sat.sum()
    Ub = sbt.sum()
    Sab = pab - MU * (sa.sum() - MU * n * n)
    Saa = sq_a - 2.0 * MU * sa.sum() + MU * MU * n * n
    Sbb = sq_b - 2.0 * MU * sb.sum() + MU * MU * n * n

    sumAB = Sab - 2.0 * np.dot(sat, sbt) / n + Ua * Ub / n ** 2
    sumAA = Saa - 2.0 * np.dot(sat, sat) / n + Ua * Ua / n ** 2
    sumBB = Sbb - 2.0 * np.dot(sbt, sbt) / n + Ub * Ub / n ** 2

    inv_n2 = 1.0 / (n * n)
    dcor = (-np.sqrt(sumAB * inv_n2)
            / np.sqrt(np.sqrt(sumAA * inv_n2) * np.sqrt(sumBB * inv_n2)))
    return np.asarray(dcor, dtype=np.float32)


def run(x, y, mm_mode=None, trace=False, tmpdir=None):
    nc = _get_program()
    in_maps, (px, py) = make_in_maps(x, y)
    res = run_bass_kernel_spmd(nc, in_maps, core_ids=list(range(NCORES)),
                               trace=trace, tmpdir=tmpdir)
    return finalize(res.results, px, py), res


def kernel(x, y):
    val, _ = run(x, y)
    return val


# revision 6
# speedup vs baseline: 1.0527x; 1.0023x over previous
"""DcorLoss kernel v3 — symmetric-triangle variant.

Same fp8-DoubleRow psum math as v2 (psum = n_i + n_j - 2 x_i.x_j via one
DR matmul stream; mu^2*I added on diagonal chunks; a = sqrt(psum)), but
exploits symmetry of the distance matrices: only 36 of 64 [128 x 1024]
cells per core are computed.

Cell decomposition: 64 row-chunks (I) x 8 col-windows (Jw). Each core c:
  - 8 "diag" cells: I = 8c+ci, window c  (rowsums only; the off-diagonal
    128-chunks inside the diagonal block pair up across cells, so
    counting rowsums once covers them exactly).
  - 28 "upper" cells from a 7-round round-robin tournament of the 8
    blocks: round pairs (m, M), rows from block m, window M; the two
    partner cores take 4 cells each. Counted twice (pab) and both
    rowsums (ACT accum) + colsums (PE f32r ones-matmul into PSUM,
    drained per round) feed the global row sums.

Engine budget per cell: ACT 2x(sqrt[128,1024]+accum) ~2.7us (bottleneck),
PE ~1.3-2.1us, DVE (STT pab + psC drains) ~1.4us.
"""

import numpy as np
import ml_dtypes

import concourse.bass as bass
import concourse.tile as tile
from concourse import bacc, mybir
from concourse.bass_utils import run_bass_kernel_spmd

P = 128
N = 8192
NCORES = 8
BLK = 1024
CI_N = 8
W = 1024
JT_N = 8
NK = 128
MU = 16.0
F8 = ml_dtypes.float8_e4m3
NCELL = 36          # 8 diag + 28 upper
NROUND = 7

_programs = {}


def _partner(c, r):
    """Round-robin circle method for 8 teams, rounds 0..6."""
    if c == 7:
        return r
    if r == c:
        return 7
    return (2 * r - c) % 7


def _schedule(c):
    """Per-core cell list: (wx_slot, win_slot, kind, round, k).

    kind: 'diag' or 'upper'. wx_slot: 0..35 into the stationary tile.
    win_slot: 0..7 into the moving tile. Mirrors on host and device.
    """
    cells = []
    for r in range(NROUND):
        for k in range(4):
            cells.append((8 + 4 * r + k, r + 1, "upper", r, k))
    # diag cells last: the kernel tail then has no colsum chain, and the
    # final round's psC drain overlaps the diag phase
    for ci in range(CI_N):
        cells.append((ci, 0, "diag", None, ci))
    return cells


def _wx_chunks(c):
    """Global row-chunk index for each of the 36 stationary slots."""
    chunks = [8 * c + s for s in range(8)]
    for r in range(NROUND):
        p = _partner(c, r)
        m = min(c, p)
        for k in range(4):
            off = k if c == m else 4 + k
            chunks.append(8 * m + off)
    return chunks


def _windows(c):
    """Global window index for each of the 8 moving slots."""
    wins = [c]
    for r in range(NROUND):
        wins.append(max(c, _partner(c, r)))
    return wins


def _build():
    dt = mybir.dt
    f32 = dt.float32
    f32r = dt.float32r
    f8 = dt.float8e4
    A = mybir.AluOpType
    AF = mybir.ActivationFunctionType
    DR = mybir.MatmulPerfMode.DoubleRow

    nc = bacc.Bacc("TRN2", target_bir_lowering=False, debug=False,
                   num_devices=NCORES)

    dWX = nc.dram_tensor("WX", [NK, 2, NCELL * P], f8,
                         kind="ExternalInput").ap()
    dWY = nc.dram_tensor("WY", [NK, 2, NCELL * P], f8,
                         kind="ExternalInput").ap()
    dMX = nc.dram_tensor("MX", [NK, 2, N], f8, kind="ExternalInput").ap()
    dMY = nc.dram_tensor("MY", [NK, 2, N], f8, kind="ExternalInput").ap()
    dEYE = nc.dram_tensor("EYE", [P, P], f8, kind="ExternalInput").ap()
    dEYW = nc.dram_tensor("EYW", [P, 4 * 512], f8, kind="ExternalInput").ap()
    dSEL = nc.dram_tensor("SEL", [P, 16], mybir.dt.float32r,
                          kind="ExternalInput").ap()
    dOUT = nc.dram_tensor("out", [P, 3 * NCELL], f32,
                          kind="ExternalOutput").ap()
    dCS = nc.dram_tensor("cs", [4, NROUND * 512], f32,
                         kind="ExternalOutput").ap()

    cells = _schedule(0)   # slot structure is identical on every core

    with tile.TileContext(nc) as tc:
        with tc.tile_pool(name="const", bufs=1) as cp, \
             tc.tile_pool(name="psum", bufs=1, space="PSUM") as pp, \
             tc.tile_pool(name="ab", bufs=4) as abp, \
             tc.tile_pool(name="trd", bufs=2) as trd:

            wx = cp.tile([NK, 2, NCELL * P], f8, tag="wx")
            wy = cp.tile([NK, 2, NCELL * P], f8, tag="wy")
            mx = cp.tile([NK, 2, N], f8, tag="mx")
            my = cp.tile([NK, 2, N], f8, tag="my")
            eye = cp.tile([P, P], f8, tag="eye")
            eyw = cp.tile([P, 4 * 512], f8, tag="eyw")
            # sel[:, 4q+j] = (j == q): ones-selector weights so colsum q
            # lands on psum partition q (matmul base partition must be 0)
            sel = cp.tile([P, 16], f32r, tag="sel")
            colsb = cp.tile([4, NROUND * 512], f32, tag="colsb")
            st = [cp.tile([P, NCELL], f32, tag=f"st{q}", name=f"st{q}")
                  for q in range(3)]

            # round-0 cells (slots 8-15, window 1) run first: ship their
            # data first, split across two queues to parallelize the
            # ~0.6us-per-DMA issue latency; diag-phase data (slots 0-7,
            # window 0, eye/eyw) is needed last
            s811 = bass.ds(8 * P, 8 * P)
            sl1 = bass.ts(1, W)
            nc.sync.dma_start(wx[:, :, s811], dWX[:, :, s811])
            nc.scalar.dma_start(wy[:, :, s811], dWY[:, :, s811])
            nc.sync.dma_start(mx[:, :, sl1], dMX[:, :, sl1])
            nc.scalar.dma_start(my[:, :, sl1], dMY[:, :, sl1])
            nc.sync.dma_start(sel[:], dSEL[:])
            for w in range(2, JT_N):
                sl = bass.ts(w, W)
                nc.sync.dma_start(mx[:, :, sl], dMX[:, :, sl])
                nc.sync.dma_start(my[:, :, sl], dMY[:, :, sl])
            srest = bass.ds(16 * P, (NCELL - 16) * P)
            nc.sync.dma_start(wx[:, :, srest], dWX[:, :, srest])
            nc.sync.dma_start(wy[:, :, srest], dWY[:, :, srest])
            s07 = bass.ds(0, 8 * P)
            sl0 = bass.ts(0, W)
            nc.sync.dma_start(wx[:, :, s07], dWX[:, :, s07])
            nc.sync.dma_start(wy[:, :, s07], dWY[:, :, s07])
            nc.sync.dma_start(mx[:, :, sl0], dMX[:, :, sl0])
            nc.sync.dma_start(my[:, :, sl0], dMY[:, :, sl0])
            nc.sync.dma_start(eye[:], dEYE[:])
            nc.sync.dma_start(eyw[:], dEYW[:])

            wz = cp.tile([2, 512], f8, tag="wz")
            nc.vector.memset(wz[:], 0.0)
            wzl = cp.tile([2, P], f8, tag="wzl")
            nc.vector.memset(wzl[:], 0.0)
            for q in range(3):
                wt = pp.tile([P, W], f32, tag="ps", bufs=3)
                for h in range(2):
                    nc.tensor.matmul(wt[:, bass.ts(h, 512)], wzl[:], wz[:],
                                     start=True, stop=True)

            pend = []       # delayed colsum work: (aT, bT, rnd, k)
            pc = {"t": None}

            def emit_colsums():
                if not pend:
                    return
                aT, bT, rnd, k = pend.pop(0)
                if k == 0:
                    pc["t"] = pp.tile([4, 512], f32, tag="pc", bufs=2,
                                      name="pc")
                pct = pc["t"]
                for q, (src, h) in enumerate(
                        ((aT, 0), (aT, 1), (bT, 0), (bT, 1))):
                    nc.tensor.matmul(
                        pct[:, :],
                        sel[:, bass.ts(q, 4)],
                        src[:, bass.ts(h, 512)],
                        start=(k == 0 and q == 0),
                        stop=(k == 3 and q == 3))
                if k == 3:
                    nc.vector.tensor_copy(
                        colsb[:, bass.ts(rnd, 512)], pct[:, :])

            for idx, (ss, ws, kind, rnd, k) in enumerate(cells):
                psA = pp.tile([P, W], f32, tag="ps", bufs=3)
                psB = pp.tile([P, W], f32, tag="ps", bufs=3)
                diag = kind == "diag"
                hd = k // 4 if not diag else k // 4  # k==ci for diag
                ci = k
                for ps_, wt_, mt_ in ((psA, wx, mx), (psB, wy, my)):
                    for h in range(2):
                        nc.tensor.matmul(
                            ps_[:, bass.ts(h, 512)],
                            wt_[:, :, bass.ts(ss, P)],
                            mt_[:, :, bass.ds(ws * W + h * 512, 512)],
                            start=True,
                            stop=not (diag and h == ci // 4),
                            perf_mode=DR)
                    if diag:
                        nc.tensor.matmul(
                            ps_[:, bass.ts(ci // 4, 512)],
                            eye[:], eyw[:, bass.ts(ci % 4, 512)],
                            start=False, stop=True)
                emit_colsums()

                aT = abp.tile([P, W], f32r, tag="ab")
                bT = abp.tile([P, W], f32r, tag="ab")
                nc.scalar.activation(aT[:], psA[:], AF.Sqrt,
                                     accum_out=st[0][:, idx:idx + 1])
                nc.scalar.activation(bT[:], psB[:], AF.Sqrt,
                                     accum_out=st[1][:, idx:idx + 1])
                t0 = trd.tile([P, W], f32, tag="r")
                nc.vector.scalar_tensor_tensor(
                    t0[:], aT[:].bitcast(f32), MU, bT[:].bitcast(f32),
                    op0=A.subtract, op1=A.mult,
                    accum_out=st[2][:, idx:idx + 1])
                if not diag:
                    pend.append((aT, bT, rnd, k))

            while pend:
                emit_colsums()

            for q in range(3):
                nc.sync.dma_start(dOUT[:, bass.ts(q, NCELL)], st[q][:])
            nc.sync.dma_start(dCS[:], colsb[:])

    nc.compile()
    return nc


def _get_program():
    if "p" not in _programs:
        _programs["p"] = _build()
    return _programs["p"]


def _f8r(a):
    return np.asarray(a, np.float64).astype(F8).astype(np.float64)


def _prep(v):
    v8 = np.asarray(v, np.float32).astype(F8)
    v8d = v8.astype(np.float64)
    assert np.all(np.isfinite(v8d))
    w8d = -2.0 * v8d
    nhat = (v8d * v8d).sum(1)
    hi = _f8r(nhat / 2.0)
    r1 = nhat - 2.0 * hi
    mid = _f8r(r1)
    r2 = r1 - mid
    lo = _f8r(r2)
    nspl = 2.0 * hi + mid + lo
    return dict(v8d=v8d, w8d=w8d, nhat=nhat, hi=hi, mid=mid, lo=lo,
                nspl=nspl)


def _pack_w(pr, row_idx):
    """Stationary tile [NK, 2, NCELL*P] for given global rows."""
    nslot = len(row_idx) // P
    WT = np.zeros((NK, 2, nslot * P), np.float64)
    wb = pr["w8d"][row_idx]
    for t in range(2):
        WT[0:64, t, :] = wb[:, t * 64:(t + 1) * 64].T
    WT[64, 0, :] = pr["hi"][row_idx]
    WT[64, 1, :] = pr["mid"][row_idx]
    WT[65, 0, :] = pr["lo"][row_idx]
    WT[65, 1, :] = 2.0
    WT[66, 0, :] = 1.0
    WT[66, 1, :] = 1.0
    return WT.astype(F8)


def _pack_m(pr, perm):
    MT = np.zeros((NK, 2, N), np.float64)
    vp = pr["v8d"][perm]
    for t in range(2):
        MT[0:64, t, :] = vp[:, t * 64:(t + 1) * 64].T
    MT[64, 0, :] = 2.0
    MT[64, 1, :] = 1.0
    MT[65, 0, :] = 1.0
    MT[65, 1, :] = pr["hi"][perm]
    MT[66, 0, :] = pr["mid"][perm]
    MT[66, 1, :] = pr["lo"][perm]
    return MT.astype(F8)


def make_in_maps(x, y):
    px = _prep(x)
    py = _prep(y)
    eye = (np.eye(P) * MU).astype(F8)
    eyw = np.zeros((P, 4 * 512), np.float64)
    for kk in range(4):
        for p in range(P):
            eyw[p, kk * 512 + kk * P + p] = MU
    eyw = eyw.astype(F8)
    selh = np.zeros((P, 16), np.float32)
    for q in range(4):
        selh[:, 5 * q] = 1.0
    in_maps = []
    for c in range(NCORES):
        chunks = _wx_chunks(c)
        row_idx = np.concatenate(
            [np.arange(I * P, (I + 1) * P) for I in chunks])
        wins = _windows(c)
        perm = np.concatenate(
            [np.arange(wv * W, (wv + 1) * W) for wv in wins])
        in_maps.append({
            "WX": _pack_w(px, row_idx),
            "WY": _pack_w(py, row_idx),
            "MX": _pack_m(px, perm),
            "MY": _pack_m(py, perm),
            "EYE": eye,
            "EYW": eyw,
            "SEL": selh,
        })
    return in_maps, (px, py)


def finalize(results, px, py):
    n = float(N)
    rs_a = np.zeros(N, np.float64)
    rs_b = np.zeros(N, np.float64)
    pab = 0.0
    for c in range(NCORES):
        o = np.asarray(results[c]["out"], np.float64)
        cs = np.asarray(results[c]["cs"], np.float64)
        cells = _schedule(c)
        chunks = _wx_chunks(c)
        wins = _windows(c)
        for idx, (ss, ws, kind, rnd, k) in enumerate(cells):
            I = chunks[ss]
            rows = slice(I * P, (I + 1) * P)
            rs_a[rows] += o[:, idx]
            rs_b[rows] += o[:, NCELL + idx]
            mult = 1.0 if kind == "diag" else 2.0
            pab += mult * o[:, 2 * NCELL + idx].sum()
        # colsums: round r covers window wins[r+1]
        for r in range(NROUND):
            wv = wins[r + 1]
            seg = cs[:, r * 512:(r + 1) * 512]
            rs_a[wv * W:wv * W + 512] += seg[0]
            rs_a[wv * W + 512:(wv + 1) * W] += seg[1]
            rs_b[wv * W:wv * W + 512] += seg[2]
            rs_b[wv * W + 512:(wv + 1) * W] += seg[3]

    def sq_closed(pr):
        sx = pr["v8d"].sum(0)
        Sg = float((sx * sx).sum())
        q_ii = 256.0 + 2.0 * (pr["nspl"] - pr["nhat"])
        return (2.0 * n * pr["nspl"].sum() - 2.0 * Sg + 256.0 * n
                - q_ii.sum())

    sq_a = sq_closed(px)
    sq_b = sq_closed(py)

    sa = rs_a - MU
    sb = rs_b - MU
    sat = sa - n * MU
    sbt = sb - n * MU
    Ua = sat.sum()
    Ub = sbt.sum()
    Sab = pab - MU * (sa.sum() - MU * n * n)
    Saa = sq_a - 2.0 * MU * sa.sum() + MU * MU * n * n
    Sbb = sq_b - 2.0 * MU * sb.sum() + MU * MU * n * n

    sumAB = Sab - 2.0 * np.dot(sat, sbt) / n + Ua * Ub / n ** 2
    sumAA = Saa - 2.0 * np.dot(sat, sat) / n + Ua * Ua / n ** 2
    sumBB = Sbb - 2.0 * np.dot(sbt, sbt) / n + Ub * Ub / n ** 2

    inv_n2 = 1.0 / (n * n)
    dcor = (-np.sqrt(sumAB * inv_n2)
            / np.sqrt(np.sqrt(sumAA * inv_n2) * np.sqrt(sumBB * inv_n2)))
    return np.asarray(dcor, dtype=np.float32)


def run(x, y, mm_mode=None, trace=False, tmpdir=None):
    nc = _get_program()
    in_maps, (px, py) = make_in_maps(x, y)
    res = run_bass_kernel_spmd(nc, in_maps, core_ids=list(range(NCORES)),
                               trace=trace, tmpdir=tmpdir)
    return finalize(res.results, px, py), res


def kernel(x, y):
    val, _ = run(x, y)
    return val


# revision 7
# speedup vs baseline: 1.0866x; 1.0322x over previous
"""DcorLoss kernel v3 — symmetric-triangle variant.

Same fp8-DoubleRow psum math as v2 (psum = n_i + n_j - 2 x_i.x_j via one
DR matmul stream; mu^2*I added on diagonal chunks; a = sqrt(psum)), but
exploits symmetry of the distance matrices: only 36 of 64 [128 x 1024]
cells per core are computed.

Cell decomposition: 64 row-chunks (I) x 8 col-windows (Jw). Each core c:
  - 8 "diag" cells: I = 8c+ci, window c  (rowsums only; the off-diagonal
    128-chunks inside the diagonal block pair up across cells, so
    counting rowsums once covers them exactly).
  - 28 "upper" cells from a 7-round round-robin tournament of the 8
    blocks: round pairs (m, M), rows from block m, window M; the two
    partner cores take 4 cells each. Counted twice (pab) and both
    rowsums (ACT accum) + colsums (PE f32r ones-matmul into PSUM,
    drained per round) feed the global row sums.

Engine budget per cell: ACT 2x(sqrt[128,1024]+accum) ~2.4us (bottleneck;
accum drains pipeline under the next ACTIVATE), PE ~2.1us, DVE (STT pab
+ psC drains) ~1.4us. All inputs are host-prepared fp8 bytes (bit-exact
replicated in the fp64 host finalize); zero on-device setup compute.
Measured: ~104-110us HW exec (vs 286us baseline), rel err 3.6e-4
(gate 2e-2). Known environmental variance: the part occasionally sits
in a slower power state for ~a minute, adding ~20%.
"""

import numpy as np
import ml_dtypes

import concourse.bass as bass
import concourse.tile as tile
from concourse import bacc, mybir
from concourse.bass_utils import run_bass_kernel_spmd

P = 128
N = 8192
NCORES = 8
BLK = 1024
CI_N = 8
W = 1024
JT_N = 8
NK = 128
MU = 16.0
F8 = ml_dtypes.float8_e4m3
NCELL = 36          # 8 diag + 28 upper
NROUND = 7

_programs = {}


def _partner(c, r):
    """Round-robin circle method for 8 teams, rounds 0..6."""
    if c == 7:
        return r
    if r == c:
        return 7
    return (2 * r - c) % 7


def _schedule(c):
    """Per-core cell list: (wx_slot, win_slot, kind, round, k).

    kind: 'diag' or 'upper'. wx_slot: 0..35 into the stationary tile.
    win_slot: 0..7 into the moving tile. Mirrors on host and device.
    """
    cells = []
    for r in range(NROUND):
        for k in range(4):
            cells.append((8 + 4 * r + k, r + 1, "upper", r, k))
    # diag cells last: the kernel tail then has no colsum chain, and the
    # final round's psC drain overlaps the diag phase
    for ci in range(CI_N):
        cells.append((ci, 0, "diag", None, ci))
    return cells


def _wx_chunks(c):
    """Global row-chunk index for each of the 36 stationary slots."""
    chunks = [8 * c + s for s in range(8)]
    for r in range(NROUND):
        p = _partner(c, r)
        m = min(c, p)
        for k in range(4):
            off = k if c == m else 4 + k
            chunks.append(8 * m + off)
    return chunks


def _windows(c):
    """Global window index for each of the 8 moving slots."""
    wins = [c]
    for r in range(NROUND):
        wins.append(max(c, _partner(c, r)))
    return wins


def _build():
    dt = mybir.dt
    f32 = dt.float32
    f32r = dt.float32r
    f8 = dt.float8e4
    A = mybir.AluOpType
    AF = mybir.ActivationFunctionType
    DR = mybir.MatmulPerfMode.DoubleRow

    nc = bacc.Bacc("TRN2", target_bir_lowering=False, debug=False,
                   num_devices=NCORES)

    dWX = nc.dram_tensor("WX", [NK, 2, NCELL * P], f8,
                         kind="ExternalInput").ap()
    dWY = nc.dram_tensor("WY", [NK, 2, NCELL * P], f8,
                         kind="ExternalInput").ap()
    dMX = nc.dram_tensor("MX", [NK, 2, N], f8, kind="ExternalInput").ap()
    dMY = nc.dram_tensor("MY", [NK, 2, N], f8, kind="ExternalInput").ap()
    dEYE = nc.dram_tensor("EYE", [P, P], f8, kind="ExternalInput").ap()
    dEYW = nc.dram_tensor("EYW", [P, 4 * 512], f8, kind="ExternalInput").ap()
    dSEL = nc.dram_tensor("SEL", [P, 16], mybir.dt.float32r,
                          kind="ExternalInput").ap()
    dOUT = nc.dram_tensor("out", [P, 3 * NCELL], f32,
                          kind="ExternalOutput").ap()
    dCS = nc.dram_tensor("cs", [4, NROUND * 512], f32,
                         kind="ExternalOutput").ap()

    cells = _schedule(0)   # slot structure is identical on every core

    with tile.TileContext(nc) as tc:
        with tc.tile_pool(name="const", bufs=1) as cp, \
             tc.tile_pool(name="psum", bufs=1, space="PSUM") as pp, \
             tc.tile_pool(name="ab", bufs=4) as abp, \
             tc.tile_pool(name="trd", bufs=2) as trd:

            wx = cp.tile([NK, 2, NCELL * P], f8, tag="wx")
            wy = cp.tile([NK, 2, NCELL * P], f8, tag="wy")
            mx = cp.tile([NK, 2, N], f8, tag="mx")
            my = cp.tile([NK, 2, N], f8, tag="my")
            eye = cp.tile([P, P], f8, tag="eye")
            eyw = cp.tile([P, 4 * 512], f8, tag="eyw")
            # sel[:, 4q+j] = (j == q): ones-selector weights so colsum q
            # lands on psum partition q (matmul base partition must be 0)
            sel = cp.tile([P, 16], f32r, tag="sel")
            colsb = cp.tile([4, NROUND * 512], f32, tag="colsb")
            st = [cp.tile([P, NCELL], f32, tag=f"st{q}", name=f"st{q}")
                  for q in range(3)]

            # round-0 cells (slots 8-15, window 1) run first: ship their
            # data first, split across two queues to parallelize the
            # ~0.6us-per-DMA issue latency; diag-phase data (slots 0-7,
            # window 0, eye/eyw) is needed last
            s811 = bass.ds(8 * P, 8 * P)
            sl1 = bass.ts(1, W)
            nc.sync.dma_start(wx[:, :, s811], dWX[:, :, s811])
            nc.scalar.dma_start(wy[:, :, s811], dWY[:, :, s811])
            nc.sync.dma_start(mx[:, :, sl1], dMX[:, :, sl1])
            nc.scalar.dma_start(my[:, :, sl1], dMY[:, :, sl1])
            nc.sync.dma_start(sel[:], dSEL[:])
            for w in range(2, JT_N):
                sl = bass.ts(w, W)
                nc.sync.dma_start(mx[:, :, sl], dMX[:, :, sl])
                nc.sync.dma_start(my[:, :, sl], dMY[:, :, sl])
            srest = bass.ds(16 * P, (NCELL - 16) * P)
            nc.sync.dma_start(wx[:, :, srest], dWX[:, :, srest])
            nc.sync.dma_start(wy[:, :, srest], dWY[:, :, srest])
            s07 = bass.ds(0, 8 * P)
            sl0 = bass.ts(0, W)
            nc.sync.dma_start(wx[:, :, s07], dWX[:, :, s07])
            nc.sync.dma_start(wy[:, :, s07], dWY[:, :, s07])
            nc.sync.dma_start(mx[:, :, sl0], dMX[:, :, sl0])
            nc.sync.dma_start(my[:, :, sl0], dMY[:, :, sl0])
            nc.sync.dma_start(eye[:], dEYE[:])
            nc.sync.dma_start(eyw[:], dEYW[:])

            wz = cp.tile([2, 512], f8, tag="wz")
            nc.vector.memset(wz[:], 0.0)
            wzl = cp.tile([2, P], f8, tag="wzl")
            nc.vector.memset(wzl[:], 0.0)
            for q in range(3):
                wt = pp.tile([P, W], f32, tag="ps", bufs=3)
                for h in range(2):
                    nc.tensor.matmul(wt[:, bass.ts(h, 512)], wzl[:], wz[:],
                                     start=True, stop=True)

            pend = []       # delayed colsum work: (aT, bT, rnd, k)
            pc = {"t": None}

            def emit_colsums():
                if not pend:
                    return
                aT, bT, rnd, k = pend.pop(0)
                if k == 0:
                    pc["t"] = pp.tile([4, 512], f32, tag="pc", bufs=2,
                                      name="pc")
                pct = pc["t"]
                for q, (src, h) in enumerate(
                        ((aT, 0), (aT, 1), (bT, 0), (bT, 1))):
                    nc.tensor.matmul(
                        pct[:, :],
                        sel[:, bass.ts(q, 4)],
                        src[:, bass.ts(h, 512)],
                        start=(k == 0 and q == 0),
                        stop=(k == 3 and q == 3))
                if k == 3:
                    nc.vector.tensor_copy(
                        colsb[:, bass.ts(rnd, 512)], pct[:, :])

            for idx, (ss, ws, kind, rnd, k) in enumerate(cells):
                psA = pp.tile([P, W], f32, tag="ps", bufs=3)
                psB = pp.tile([P, W], f32, tag="ps", bufs=3)
                diag = kind == "diag"
                hd = k // 4 if not diag else k // 4  # k==ci for diag
                ci = k
                for ps_, wt_, mt_ in ((psA, wx, mx), (psB, wy, my)):
                    for h in range(2):
                        nc.tensor.matmul(
                            ps_[:, bass.ts(h, 512)],
                            wt_[:, :, bass.ts(ss, P)],
                            mt_[:, :, bass.ds(ws * W + h * 512, 512)],
                            start=True,
                            stop=not (diag and h == ci // 4),
                            perf_mode=DR)
                    if diag:
                        nc.tensor.matmul(
                            ps_[:, bass.ts(ci // 4, 512)],
                            eye[:], eyw[:, bass.ts(ci % 4, 512)],
                            start=False, stop=True)
                emit_colsums()

                aT = abp.tile([P, W], f32r, tag="ab")
                bT = abp.tile([P, W], f32r, tag="ab")
                nc.scalar.activation(aT[:], psA[:], AF.Sqrt,
                                     accum_out=st[0][:, idx:idx + 1])
                nc.scalar.activation(bT[:], psB[:], AF.Sqrt,
                                     accum_out=st[1][:, idx:idx + 1])
                t0 = trd.tile([P, W], f32, tag="r")
                nc.vector.scalar_tensor_tensor(
                    t0[:], aT[:].bitcast(f32), MU, bT[:].bitcast(f32),
                    op0=A.subtract, op1=A.mult,
                    accum_out=st[2][:, idx:idx + 1])
                if not diag:
                    pend.append((aT, bT, rnd, k))

            while pend:
                emit_colsums()

            for q in range(3):
                nc.sync.dma_start(dOUT[:, bass.ts(q, NCELL)], st[q][:])
            nc.sync.dma_start(dCS[:], colsb[:])

    nc.compile()
    return nc


def _get_program():
    if "p" not in _programs:
        _programs["p"] = _build()
    return _programs["p"]


def _f8r(a):
    return np.asarray(a, np.float64).astype(F8).astype(np.float64)


def _prep(v):
    v8 = np.asarray(v, np.float32).astype(F8)
    v8d = v8.astype(np.float64)
    assert np.all(np.isfinite(v8d))
    w8d = -2.0 * v8d
    nhat = (v8d * v8d).sum(1)
    hi = _f8r(nhat / 2.0)
    r1 = nhat - 2.0 * hi
    mid = _f8r(r1)
    r2 = r1 - mid
    lo = _f8r(r2)
    nspl = 2.0 * hi + mid + lo
    return dict(v8d=v8d, w8d=w8d, nhat=nhat, hi=hi, mid=mid, lo=lo,
                nspl=nspl)


def _pack_w(pr, row_idx):
    """Stationary tile [NK, 2, NCELL*P] for given global rows."""
    nslot = len(row_idx) // P
    WT = np.zeros((NK, 2, nslot * P), np.float64)
    wb = pr["w8d"][row_idx]
    for t in range(2):
        WT[0:64, t, :] = wb[:, t * 64:(t + 1) * 64].T
    WT[64, 0, :] = pr["hi"][row_idx]
    WT[64, 1, :] = pr["mid"][row_idx]
    WT[65, 0, :] = pr["lo"][row_idx]
    WT[65, 1, :] = 2.0
    WT[66, 0, :] = 1.0
    WT[66, 1, :] = 1.0
    return WT.astype(F8)


def _pack_m(pr, perm):
    MT = np.zeros((NK, 2, N), np.float64)
    vp = pr["v8d"][perm]
    for t in range(2):
        MT[0:64, t, :] = vp[:, t * 64:(t + 1) * 64].T
    MT[64, 0, :] = 2.0
    MT[64, 1, :] = 1.0
    MT[65, 0, :] = 1.0
    MT[65, 1, :] = pr["hi"][perm]
    MT[66, 0, :] = pr["mid"][perm]
    MT[66, 1, :] = pr["lo"][perm]
    return MT.astype(F8)


def make_in_maps(x, y):
    px = _prep(x)
    py = _prep(y)
    eye = (np.eye(P) * MU).astype(F8)
    eyw = np.zeros((P, 4 * 512), np.float64)
    for kk in range(4):
        for p in range(P):
            eyw[p, kk * 512 + kk * P + p] = MU
    eyw = eyw.astype(F8)
    selh = np.zeros((P, 16), np.float32)
    for q in range(4):
        selh[:, 5 * q] = 1.0
    in_maps = []
    for c in range(NCORES):
        chunks = _wx_chunks(c)
        row_idx = np.concatenate(
            [np.arange(I * P, (I + 1) * P) for I in chunks])
        wins = _windows(c)
        perm = np.concatenate(
            [np.arange(wv * W, (wv + 1) * W) for wv in wins])
        in_maps.append({
            "WX": _pack_w(px, row_idx),
            "WY": _pack_w(py, row_idx),
            "MX": _pack_m(px, perm),
            "MY": _pack_m(py, perm),
            "EYE": eye,
            "EYW": eyw,
            "SEL": selh,
        })
    return in_maps, (px, py)


def finalize(results, px, py):
    n = float(N)
    rs_a = np.zeros(N, np.float64)
    rs_b = np.zeros(N, np.float64)
    pab = 0.0
    for c in range(NCORES):
        o = np.asarray(results[c]["out"], np.float64)
        cs = np.asarray(results[c]["cs"], np.float64)
        cells = _schedule(c)
        chunks = _wx_chunks(c)
        wins = _windows(c)
        for idx, (ss, ws, kind, rnd, k) in enumerate(cells):
            I = chunks[ss]
            rows = slice(I * P, (I + 1) * P)
            rs_a[rows] += o[:, idx]
            rs_b[rows] += o[:, NCELL + idx]
            mult = 1.0 if kind == "diag" else 2.0
            pab += mult * o[:, 2 * NCELL + idx].sum()
        # colsums: round r covers window wins[r+1]
        for r in range(NROUND):
            wv = wins[r + 1]
            seg = cs[:, r * 512:(r + 1) * 512]
            rs_a[wv * W:wv * W + 512] += seg[0]
            rs_a[wv * W + 512:(wv + 1) * W] += seg[1]
            rs_b[wv * W:wv * W + 512] += seg[2]
            rs_b[wv * W + 512:(wv + 1) * W] += seg[3]

    def sq_closed(pr):
        sx = pr["v8d"].sum(0)
        Sg = float((sx * sx).sum())
        q_ii = 256.0 + 2.0 * (pr["nspl"] - pr["nhat"])
        return (2.0 * n * pr["nspl"].sum() - 2.0 * Sg + 256.0 * n
                - q_ii.sum())

    sq_a = sq_closed(px)
    sq_b = sq_closed(py)

    sa = rs_a - MU
    sb = rs_b - MU
    sat = sa - n * MU
    sbt = sb - n * MU
    Ua = sat.sum()
    Ub = sbt.sum()
    Sab = pab - MU * (sa.sum() - MU * n * n)
    Saa = sq_a - 2.0 * MU * sa.sum() + MU * MU * n * n
    Sbb = sq_b - 2.0 * MU * sb.sum() + MU * MU * n * n

    sumAB = Sab - 2.0 * np.dot(sat, sbt) / n + Ua * Ub / n ** 2
    sumAA = Saa - 2.0 * np.dot(sat, sat) / n + Ua * Ua / n ** 2
    sumBB = Sbb - 2.0 * np.dot(sbt, sbt) / n + Ub * Ub / n ** 2

    inv_n2 = 1.0 / (n * n)
    dcor = (-np.sqrt(sumAB * inv_n2)
            / np.sqrt(np.sqrt(sumAA * inv_n2) * np.sqrt(sumBB * inv_n2)))
    return np.asarray(dcor, dtype=np.float32)


def run(x, y, mm_mode=None, trace=False, tmpdir=None):
    nc = _get_program()
    in_maps, (px, py) = make_in_maps(x, y)
    res = run_bass_kernel_spmd(nc, in_maps, core_ids=list(range(NCORES)),
                               trace=trace, tmpdir=tmpdir)
    return finalize(res.results, px, py), res


def kernel(x, y):
    val, _ = run(x, y)
    return val


# revision 8
# speedup vs baseline: 1.0869x; 1.0002x over previous
"""DcorLoss kernel v3 — symmetric-triangle variant.

Same fp8-DoubleRow psum math as v2 (psum = n_i + n_j - 2 x_i.x_j via one
DR matmul stream; mu^2*I added on diagonal chunks; a = sqrt(psum)), but
exploits symmetry of the distance matrices: only 36 of 64 [128 x 1024]
cells per core are computed.

Cell decomposition: 64 row-chunks (I) x 8 col-windows (Jw). Each core c:
  - 8 "diag" cells: I = 8c+ci, window c  (rowsums only; the off-diagonal
    128-chunks inside the diagonal block pair up across cells, so
    counting rowsums once covers them exactly).
  - 28 "upper" cells from a 7-round round-robin tournament of the 8
    blocks: round pairs (m, M), rows from block m, window M; the two
    partner cores take 4 cells each. Counted twice (pab) and both
    rowsums (ACT accum) + colsums (PE f32r ones-matmul into PSUM,
    drained per round) feed the global row sums.

Engine budget per cell: ACT 2x(sqrt[128,1024]+accum) ~2.7us (bottleneck),
PE ~1.3-2.1us, DVE (STT pab + psC drains) ~1.4us.
"""

import numpy as np
import ml_dtypes

import concourse.bass as bass
import concourse.tile as tile
from concourse import bacc, mybir
from concourse.bass_utils import run_bass_kernel_spmd

P = 128
N = 8192
NCORES = 8
BLK = 1024
CI_N = 8
W = 1024
JT_N = 8
NK = 128
MU = 16.0
F8 = ml_dtypes.float8_e4m3
NCELL = 36          # 8 diag + 28 upper
NROUND = 7

_programs = {}


def _partner(c, r):
    """Round-robin circle method for 8 teams, rounds 0..6."""
    if c == 7:
        return r
    if r == c:
        return 7
    return (2 * r - c) % 7


def _schedule(c):
    """Per-core cell list: (wx_slot, win_slot, kind, round, k).

    kind: 'diag' or 'upper'. wx_slot: 0..35 into the stationary tile.
    win_slot: 0..7 into the moving tile. Mirrors on host and device.
    """
    cells = []
    for r in range(NROUND):
        for k in range(4):
            cells.append((8 + 4 * r + k, r + 1, "upper", r, k))
    # diag cells last: the kernel tail then has no colsum chain, and the
    # final round's psC drain overlaps the diag phase
    for ci in range(CI_N):
        cells.append((ci, 0, "diag", None, ci))
    return cells


def _wx_chunks(c):
    """Global row-chunk index for each of the 36 stationary slots."""
    chunks = [8 * c + s for s in range(8)]
    for r in range(NROUND):
        p = _partner(c, r)
        m = min(c, p)
        for k in range(4):
            off = k if c == m else 4 + k
            chunks.append(8 * m + off)
    return chunks


def _windows(c):
    """Global window index for each of the 8 moving slots."""
    wins = [c]
    for r in range(NROUND):
        wins.append(max(c, _partner(c, r)))
    return wins


def _build():
    dt = mybir.dt
    f32 = dt.float32
    f32r = dt.float32r
    f8 = dt.float8e4
    A = mybir.AluOpType
    AF = mybir.ActivationFunctionType
    DR = mybir.MatmulPerfMode.DoubleRow

    nc = bacc.Bacc("TRN2", target_bir_lowering=False, debug=False,
                   num_devices=NCORES)

    dWX = nc.dram_tensor("WX", [NK, 2, NCELL * P], f8,
                         kind="ExternalInput").ap()
    dWY = nc.dram_tensor("WY", [NK, 2, NCELL * P], f8,
                         kind="ExternalInput").ap()
    dMX = nc.dram_tensor("MX", [NK, 2, N], f8, kind="ExternalInput").ap()
    dMY = nc.dram_tensor("MY", [NK, 2, N], f8, kind="ExternalInput").ap()
    dEYE = nc.dram_tensor("EYE", [P, P], f8, kind="ExternalInput").ap()
    dEYW = nc.dram_tensor("EYW", [P, 4 * 512], f8, kind="ExternalInput").ap()
    dSEL = nc.dram_tensor("SEL", [P, 16], mybir.dt.float32r,
                          kind="ExternalInput").ap()
    dOUT = nc.dram_tensor("out", [P, 3 * NCELL + 4], f32,
                          kind="ExternalOutput").ap()
    dCS = nc.dram_tensor("cs", [4, (NROUND + 1) * 512], f32,
                         kind="ExternalOutput").ap()

    cells = _schedule(0)   # slot structure is identical on every core

    with tile.TileContext(nc) as tc:
        with tc.tile_pool(name="const", bufs=1) as cp, \
             tc.tile_pool(name="psum", bufs=1, space="PSUM") as pp, \
             tc.tile_pool(name="ab", bufs=4) as abp, \
             tc.tile_pool(name="trd", bufs=2) as trd:

            wx = cp.tile([NK, 2, NCELL * P], f8, tag="wx")
            wy = cp.tile([NK, 2, NCELL * P], f8, tag="wy")
            mx = cp.tile([NK, 2, N], f8, tag="mx")
            my = cp.tile([NK, 2, N], f8, tag="my")
            eye = cp.tile([P, P], f8, tag="eye")
            eyw = cp.tile([P, 4 * 512], f8, tag="eyw")
            # sel[:, 4q+j] = (j == q): ones-selector weights so colsum q
            # lands on psum partition q (matmul base partition must be 0)
            sel = cp.tile([P, 16], f32r, tag="sel")
            colsb = cp.tile([4, (NROUND + 1) * 512], f32, tag="colsb")
            st = [cp.tile([P, NCELL + 4], f32, tag=f"st{q}",
                          name=f"st{q}")
                  for q in range(3)]

            # round-0 cells (slots 8-15, window 1) run first: ship their
            # data first, split across two queues to parallelize the
            # ~0.6us-per-DMA issue latency; diag-phase data (slots 0-7,
            # window 0, eye/eyw) is needed last
            s811 = bass.ds(8 * P, 8 * P)
            sl1 = bass.ts(1, W)
            nc.sync.dma_start(wx[:, :, s811], dWX[:, :, s811])
            nc.scalar.dma_start(wy[:, :, s811], dWY[:, :, s811])
            nc.sync.dma_start(mx[:, :, sl1], dMX[:, :, sl1])
            nc.scalar.dma_start(my[:, :, sl1], dMY[:, :, sl1])
            nc.sync.dma_start(sel[:], dSEL[:])
            for w in range(2, JT_N):
                sl = bass.ts(w, W)
                nc.sync.dma_start(mx[:, :, sl], dMX[:, :, sl])
                nc.sync.dma_start(my[:, :, sl], dMY[:, :, sl])
            srest = bass.ds(16 * P, (NCELL - 16) * P)
            nc.sync.dma_start(wx[:, :, srest], dWX[:, :, srest])
            nc.sync.dma_start(wy[:, :, srest], dWY[:, :, srest])
            s07 = bass.ds(0, 8 * P)
            sl0 = bass.ts(0, W)
            nc.sync.dma_start(wx[:, :, s07], dWX[:, :, s07])
            nc.sync.dma_start(wy[:, :, s07], dWY[:, :, s07])
            nc.sync.dma_start(mx[:, :, sl0], dMX[:, :, sl0])
            nc.sync.dma_start(my[:, :, sl0], dMY[:, :, sl0])
            nc.sync.dma_start(eye[:], dEYE[:])
            nc.sync.dma_start(eyw[:], dEYW[:])

            wz = cp.tile([2, 512], f8, tag="wz")
            nc.vector.memset(wz[:], 0.0)
            wzl = cp.tile([2, P], f8, tag="wzl")
            nc.vector.memset(wzl[:], 0.0)
            for q in range(3):
                wt = pp.tile([P, W], f32, tag="ps", bufs=3)
                for h in range(2):
                    nc.tensor.matmul(wt[:, bass.ts(h, 512)], wzl[:], wz[:],
                                     start=True, stop=True)

            pend = []       # delayed colsum work: (aT, bT, rnd, k)
            pc = {"t": None}

            def emit_colsums():
                if not pend:
                    return
                aT, bT, rnd, k = pend.pop(0)
                if k == 0:
                    pc["t"] = pp.tile([4, 512], f32, tag="pc", bufs=2,
                                      name="pc")
                pct = pc["t"]
                if rnd == NROUND:
                    # diag cells 0-3: colsums of the h1 halves only (a on
                    # psum partition 0, b on partition 1)
                    work = ((aT, 1, 0), (bT, 1, 1))
                else:
                    work = ((aT, 0, 0), (aT, 1, 1), (bT, 0, 2), (bT, 1, 3))
                last_q = work[-1][2]
                for src, h, q in work:
                    nc.tensor.matmul(
                        pct[:, :],
                        sel[:, bass.ts(q, 4)],
                        src[:, bass.ts(h, 512)],
                        start=(k == 0 and q == 0),
                        stop=(k == 3 and q == last_q))
                if k == 3:
                    nc.vector.tensor_copy(
                        colsb[:, bass.ts(rnd, 512)], pct[:, :])

            for idx, (ss, ws, kind, rnd, k) in enumerate(cells):
                psA = pp.tile([P, W], f32, tag="ps", bufs=3)
                psB = pp.tile([P, W], f32, tag="ps", bufs=3)
                diag = kind == "diag"
                ci = k
                trim = diag and ci >= 4      # only cols 512-1023 needed
                hs = (1,) if trim else (0, 1)
                sl = bass.ds(512, 512) if trim else bass.ds(0, W)
                for ps_, wt_, mt_ in ((psA, wx, mx), (psB, wy, my)):
                    for h in hs:
                        nc.tensor.matmul(
                            ps_[:, bass.ts(h, 512)],
                            wt_[:, :, bass.ts(ss, P)],
                            mt_[:, :, bass.ds(ws * W + h * 512, 512)],
                            start=True,
                            stop=not (diag and h == ci // 4),
                            perf_mode=DR)
                    if diag:
                        nc.tensor.matmul(
                            ps_[:, bass.ts(ci // 4, 512)],
                            eye[:], eyw[:, bass.ts(ci % 4, 512)],
                            start=False, stop=True)
                emit_colsums()

                aT = abp.tile([P, W], f32r, tag="ab")
                bT = abp.tile([P, W], f32r, tag="ab")
                nc.scalar.activation(aT[:, sl], psA[:, sl], AF.Sqrt,
                                     accum_out=st[0][:, idx:idx + 1])
                nc.scalar.activation(bT[:, sl], psB[:, sl], AF.Sqrt,
                                     accum_out=st[1][:, idx:idx + 1])
                t0 = trd.tile([P, W], f32, tag="r")
                nc.vector.scalar_tensor_tensor(
                    t0[:, sl], aT[:, sl].bitcast(f32), MU,
                    bT[:, sl].bitcast(f32),
                    op0=A.subtract, op1=A.mult,
                    accum_out=st[2][:, idx:idx + 1])
                if diag and ci < 4:
                    # h1 part counted twice in pab (its transpose in the
                    # trimmed cells is not computed): extra h1-only accum
                    t1 = trd.tile([P, 512], f32, tag="r2", name="t1")
                    h1 = bass.ds(512, 512)
                    nc.vector.scalar_tensor_tensor(
                        t1[:], aT[:, h1].bitcast(f32), MU,
                        bT[:, h1].bitcast(f32),
                        op0=A.subtract, op1=A.mult,
                        accum_out=st[2][:, NCELL + ci:NCELL + ci + 1])
                    pend.append((aT, bT, NROUND, ci))
                if not diag:
                    pend.append((aT, bT, rnd, k))

            while pend:
                emit_colsums()

            nc.sync.dma_start(dOUT[:, 0:NCELL], st[0][:, 0:NCELL])
            nc.sync.dma_start(dOUT[:, NCELL:2 * NCELL],
                              st[1][:, 0:NCELL])
            nc.sync.dma_start(dOUT[:, 2 * NCELL:3 * NCELL + 4],
                              st[2][:, 0:NCELL + 4])
            nc.sync.dma_start(dCS[:], colsb[:])

    nc.compile()
    return nc


def _get_program():
    if "p" not in _programs:
        _programs["p"] = _build()
    return _programs["p"]


def _f8r(a):
    return np.asarray(a, np.float64).astype(F8).astype(np.float64)


def _prep(v):
    v8 = np.asarray(v, np.float32).astype(F8)
    v8d = v8.astype(np.float64)
    assert np.all(np.isfinite(v8d))
    w8d = -2.0 * v8d
    nhat = (v8d * v8d).sum(1)
    hi = _f8r(nhat / 2.0)
    r1 = nhat - 2.0 * hi
    mid = _f8r(r1)
    r2 = r1 - mid
    lo = _f8r(r2)
    nspl = 2.0 * hi + mid + lo
    return dict(v8d=v8d, w8d=w8d, nhat=nhat, hi=hi, mid=mid, lo=lo,
                nspl=nspl)


def _pack_w(pr, row_idx):
    """Stationary tile [NK, 2, NCELL*P] for given global rows."""
    nslot = len(row_idx) // P
    WT = np.zeros((NK, 2, nslot * P), np.float64)
    wb = pr["w8d"][row_idx]
    for t in range(2):
        WT[0:64, t, :] = wb[:, t * 64:(t + 1) * 64].T
    WT[64, 0, :] = pr["hi"][row_idx]
    WT[64, 1, :] = pr["mid"][row_idx]
    WT[65, 0, :] = pr["lo"][row_idx]
    WT[65, 1, :] = 2.0
    WT[66, 0, :] = 1.0
    WT[66, 1, :] = 1.0
    return WT.astype(F8)


def _pack_m(pr, perm):
    MT = np.zeros((NK, 2, N), np.float64)
    vp = pr["v8d"][perm]
    for t in range(2):
        MT[0:64, t, :] = vp[:, t * 64:(t + 1) * 64].T
    MT[64, 0, :] = 2.0
    MT[64, 1, :] = 1.0
    MT[65, 0, :] = 1.0
    MT[65, 1, :] = pr["hi"][perm]
    MT[66, 0, :] = pr["mid"][perm]
    MT[66, 1, :] = pr["lo"][perm]
    return MT.astype(F8)


def make_in_maps(x, y):
    px = _prep(x)
    py = _prep(y)
    eye = (np.eye(P) * MU).astype(F8)
    eyw = np.zeros((P, 4 * 512), np.float64)
    for kk in range(4):
        for p in range(P):
            eyw[p, kk * 512 + kk * P + p] = MU
    eyw = eyw.astype(F8)
    selh = np.zeros((P, 16), np.float32)
    for q in range(4):
        selh[:, 5 * q] = 1.0
    in_maps = []
    for c in range(NCORES):
        chunks = _wx_chunks(c)
        row_idx = np.concatenate(
            [np.arange(I * P, (I + 1) * P) for I in chunks])
        wins = _windows(c)
        perm = np.concatenate(
            [np.arange(wv * W, (wv + 1) * W) for wv in wins])
        in_maps.append({
            "WX": _pack_w(px, row_idx),
            "WY": _pack_w(py, row_idx),
            "MX": _pack_m(px, perm),
            "MY": _pack_m(py, perm),
            "EYE": eye,
            "EYW": eyw,
            "SEL": selh,
        })
    return in_maps, (px, py)


def finalize(results, px, py):
    n = float(N)
    rs_a = np.zeros(N, np.float64)
    rs_b = np.zeros(N, np.float64)
    pab = 0.0
    for c in range(NCORES):
        o = np.asarray(results[c]["out"], np.float64)
        cs = np.asarray(results[c]["cs"], np.float64)
        cells = _schedule(c)
        chunks = _wx_chunks(c)
        wins = _windows(c)
        for idx, (ss, ws, kind, rnd, k) in enumerate(cells):
            I = chunks[ss]
            rows = slice(I * P, (I + 1) * P)
            rs_a[rows] += o[:, idx]
            rs_b[rows] += o[:, NCELL + idx]
            mult = 1.0 if kind == "diag" else 2.0
            pab += mult * o[:, 2 * NCELL + idx].sum()
        # diag 0-3 h1 parts count twice in pab (transpose not computed)
        pab += o[:, 3 * NCELL:3 * NCELL + 4].sum()
        # colsums: round r covers window wins[r+1]
        for r in range(NROUND):
            wv = wins[r + 1]
            seg = cs[:, r * 512:(r + 1) * 512]
            rs_a[wv * W:wv * W + 512] += seg[0]
            rs_a[wv * W + 512:(wv + 1) * W] += seg[1]
            rs_b[wv * W:wv * W + 512] += seg[2]
            rs_b[wv * W + 512:(wv + 1) * W] += seg[3]
        # diag-block patch: trimmed cells 4-7 miss cols [0,512); add the
        # colsums of cells 0-3's h1 halves (their transposes)
        segd = cs[:, NROUND * 512:(NROUND + 1) * 512]
        rs_a[c * W + 512:(c + 1) * W] += segd[0]
        rs_b[c * W + 512:(c + 1) * W] += segd[1]

    def sq_closed(pr):
        sx = pr["v8d"].sum(0)
        Sg = float((sx * sx).sum())
        q_ii = 256.0 + 2.0 * (pr["nspl"] - pr["nhat"])
        return (2.0 * n * pr["nspl"].sum() - 2.0 * Sg + 256.0 * n
                - q_ii.sum())

    sq_a = sq_closed(px)
    sq_b = sq_closed(py)

    sa = rs_a - MU
    sb = rs_b - MU
    sat = sa - n * MU
    sbt = sb - n * MU
    Ua = sat.sum()
    Ub = sbt.sum()
    Sab = pab - MU * (sa.sum() - MU * n * n)
    Saa = sq_a - 2.0 * MU * sa.sum() + MU * MU * n * n
    Sbb = sq_b - 2.0 * MU * sb.sum() + MU * MU * n * n

    sumAB = Sab - 2.0 * np.dot(sat, sbt) / n + Ua * Ub / n ** 2
    sumAA = Saa - 2.0 * np.dot(sat, sat) / n + Ua * Ua / n ** 2
    sumBB = Sbb - 2.0 * np.dot(sbt, sbt) / n + Ub * Ub / n ** 2

    inv_n2 = 1.0 / (n * n)
    dcor = (-np.sqrt(sumAB * inv_n2)
            / np.sqrt(np.sqrt(sumAA * inv_n2) * np.sqrt(sumBB * inv_n2)))
    return np.asarray(dcor, dtype=np.float32)


def run(x, y, mm_mode=None, trace=False, tmpdir=None):
    nc = _get_program()
    in_maps, (px, py) = make_in_maps(x, y)
    res = run_bass_kernel_spmd(nc, in_maps, core_ids=list(range(NCORES)),
                               trace=trace, tmpdir=tmpdir)
    return finalize(res.results, px, py), res


def kernel(x, y):
    val, _ = run(x, y)
    return val
